# revision 37
# baseline (speedup 1.0000x reference)
"""Trainium2 Bass kernel for nn_Attention_83004537963197.

LayerNorm -> QKV projection -> 8-head attention (head_dim=16) -> output
projection, x[16, 1024, 1024] f32.  Data-parallel over batch: 2 batches
per NeuronCore across 8 cores, no collectives.

Per-core dataflow (per batch):
  A. Load x row tiles [128, 1024], LayerNorm along free dim (bn_stats),
     normalize to bf16, transpose via PE matmul against a constant
     identity (normal matmul mode - ~2x faster than transpose mode).
  B. q^T/k^T compact [128(f), n] via matmul with gamma/SCALE-folded
     weights, then SBUF->SBUF DMA relocation of each head's 16 rows to
     32-aligned "region" layout (4 heads per region at offsets 32c).
     v in row layout per (j-tile, head) as [128, 32]: col 0 = 1.0
     (softmax rowsum trick), cols 1..16 = v, rest 0.
  C. Per head-pair: scores S^T[j,i] = k_h^T.T @ q_h^T (K=16, row-tiled
     via tile_position), exp on ScalarE (PSUM -> SBUF bf16), attn@v as
     out^T[d,i] += v_aug.T @ E^T (K=128, col-tiled, N=512 - one
     accumulation group per PSUM bank).  The ones column gives softmax
     row sums at out^T row 32c; reciprocal computed on a [128, 32]
     reshape (all lanes), replicated across head rows by partition-
     stride-0 DMA from DRAM, applied with one tensor_mul -> o^T bf16.
     Row 32c becomes exactly 1.0; region 0 row 0 pairs with b_proj in
     w_proj_pad row 0 to add the bias for free.
  D. Projection with zero-padded w_proj rows (K spans the region incl.
     zeroed junk rows).

Emission is software-pipelined across the 2 batches: batch b+1's
LN/qkv/v chunks and batch b's projection chunks are emitted between
attention groups of the current batch, so the in-order PE/DVE queues
overlap them with the ScalarE-bound exp phase.
"""

from contextlib import ExitStack

import numpy as np
import ml_dtypes

import concourse.bass as bass
import concourse.tile as tile
from concourse import bacc, mybir
from concourse.bass_utils import run_bass_kernel_spmd

F32 = mybir.dt.float32
BF16 = mybir.dt.bfloat16

B, N, EMB = 16, 1024, 1024
HEADS, INNER = 8, 128
HD = INNER // HEADS            # 16
SCALE = INNER ** -0.5
EPS = 1e-5
NCORES = 8
NB = B // NCORES               # batches per core
P = 128
NT = EMB // P                  # 8 tiles along emb / n

Sub = mybir.AluOpType.subtract
Mult = mybir.AluOpType.mult
Add = mybir.AluOpType.add
AF = mybir.ActivationFunctionType

_CACHE = {}


def _build():
    nc = bacc.Bacc(None, target_bir_lowering=False)

    xs_h = nc.declare_dram_parameter("xs", [NB, N, EMB], F32, isOutput=False)
    wqk_h = nc.declare_dram_parameter("wqk", [P, NT, 2, P], BF16, isOutput=False)
    bqk_h = nc.declare_dram_parameter("bqk", [P, 2], F32, isOutput=False)
    wv_h = nc.declare_dram_parameter("wv", [P, NT, P], BF16, isOutput=False)
    bv_h = nc.declare_dram_parameter("bv", [1, P], BF16, isOutput=False)
    wpj_h = nc.declare_dram_parameter("wproj", [P, 2, EMB], BF16, isOutput=False)
    id_h = nc.declare_dram_parameter("ident", [P, P], BF16, isOutput=False)
    out_h = nc.declare_dram_parameter("out", [NB, N, EMB], F32, isOutput=True)

    with tile.TileContext(nc) as tc, ExitStack() as ctx:
        ent = ctx.enter_context
        const = ent(tc.tile_pool(name="const", bufs=1))
        xpool = ent(tc.tile_pool(name="xpool", bufs=3))
        stat = ent(tc.tile_pool(name="stat", bufs=8))
        xT_pool = ent(tc.tile_pool(name="xT", bufs=2))
        qk_pool = ent(tc.tile_pool(name="qk", bufs=2))
        v_pool = ent(tc.tile_pool(name="vp", bufs=2))
        e_pool = ent(tc.tile_pool(name="ep", bufs=4))
        o_pool = ent(tc.tile_pool(name="op", bufs=4))
        nrm_pool = ent(tc.tile_pool(name="nrm", bufs=2))
        fin_pool = ent(tc.tile_pool(name="fin", bufs=4))
        dram_pool = ent(tc.tile_pool(name="dsc", bufs=2, space="DRAM"))
        ps_small = ent(tc.tile_pool(name="pss", bufs=2, space="PSUM"))
        ps_scores = ent(tc.tile_pool(name="psc", bufs=2, space="PSUM"))
        ps_out = ent(tc.tile_pool(name="pso", bufs=2, space="PSUM"))

        # ---- constants ----
        wqk_sb = const.tile([P, NT, 2, P], BF16)
        nc.sync.dma_start(out=wqk_sb, in_=wqk_h[:])
        bqk_sb = const.tile([P, 2], F32)
        nc.sync.dma_start(out=bqk_sb, in_=bqk_h[:])
        wv_sb = const.tile([P, NT, P], BF16)
        nc.sync.dma_start(out=wv_sb, in_=wv_h[:])
        bv_sb = const.tile([1, P], BF16)
        nc.sync.dma_start(out=bv_sb, in_=bv_h[:])
        wpj_sb = const.tile([P, 2, EMB], BF16)
        nc.sync.dma_start(out=wpj_sb, in_=wpj_h[:])
        id_sb = const.tile([P, P], BF16)
        nc.sync.dma_start(out=id_sb, in_=id_h[:])
        eps_sb = const.tile([P, 1], F32)
        nc.vector.memset(eps_sb, EPS)
        ones1_sb = const.tile([1, P], BF16)
        nc.vector.memset(ones1_sb, 1.0)

        st8 = {}   # per-batch live tiles

        def emit_ln_pre(b, it):
            s = st8[b]
            if s.get("xT") is None:
                s["xT"] = xT_pool.tile([P, NT, N], BF16, tag="xTt", name="xTt")
                s["mv"] = [None] * NT
                s["xn"] = [None] * NT
                s["rs4"] = [None, None]
            xt = xpool.tile([P, EMB], F32, tag="xt", bufs=5)
            nc.sync.dma_start(out=xt, in_=xs_h[b, it * P:(it + 1) * P, :])
            st = stat.tile([P, 2, 6], F32, tag="st")
            nc.vector.bn_stats(out=st[:, 0, :], in_=xt[:, 0:512])
            nc.vector.bn_stats(out=st[:, 1, :], in_=xt[:, 512:1024])
            mv = stat.tile([P, 2], F32, tag="mv", bufs=10)
            nc.vector.bn_aggr(out=mv, in_=st)
            s["mv"][it] = mv
            g, k = it // 4, it % 4
            if s.get("var4_%d" % g) is None:
                s["var4_%d" % g] = stat.tile([P, 4], F32, tag="var4", bufs=4,
                                             name="var4")
            nc.vector.tensor_copy(out=s["var4_%d" % g][:, k:k + 1],
                                  in_=mv[:, 1:2])
            xn = xpool.tile([P, EMB], BF16, tag="xn", bufs=10)
            nc.vector.tensor_scalar(
                out=xn, in0=xt, scalar1=mv[:, 0:1], scalar2=None, op0=Sub)
            s["xn"][it] = xn

        def emit_newton(b, g):
            # rs = rsqrt(var + eps) for 4 row tiles, pure DVE (no ACT
            # table): magic-seed + 2 Newton iterations, rel err ~4e-6.
            s = st8[b]
            var4 = s["var4_%d" % g]
            ha = stat.tile([P, 4], F32, tag="nt_ha")
            nc.vector.tensor_scalar(out=ha, in0=var4, scalar1=EPS,
                                    scalar2=0.5, op0=Add, op1=Mult)
            a = stat.tile([P, 4], F32, tag="nt_a")
            nc.vector.tensor_scalar(out=a, in0=var4, scalar1=EPS,
                                    scalar2=None, op0=Add)
            tu = stat.tile([P, 4], mybir.dt.uint32, tag="nt_t")
            nc.vector.tensor_scalar(
                out=tu, in0=a.bitcast(mybir.dt.uint32), scalar1=1,
                scalar2=None, op0=mybir.AluOpType.logical_shift_right)
            mg = stat.tile([P, 4], mybir.dt.uint32, tag="nt_mg")
            nc.vector.memset(mg, 0x5f3759df)
            y0b = stat.tile([P, 4], mybir.dt.uint32, tag="nt_y0")
            nc.vector.tensor_tensor(out=y0b, in0=mg, in1=tu,
                                    op=mybir.AluOpType.subtract)
            y = y0b.bitcast(F32)
            for _ in range(2):
                y2 = stat.tile([P, 4], F32, tag="nt_y2")
                nc.vector.tensor_mul(y2, y, y)
                nc.vector.tensor_mul(y2, y2, ha)
                nc.vector.tensor_scalar(out=y2, in0=y2, scalar1=-1.0,
                                        scalar2=1.5, op0=Mult, op1=Add)
                yn = stat.tile([P, 4], F32, tag="nt_yn")
                nc.vector.tensor_mul(yn, y, y2)
                y = yn
            s["rs4"][g] = y

        def emit_ln_post(b, it):
            s = st8[b]
            xT = s["xT"]
            g, k = it // 4, it % 4
            rs = s["rs4"][g][:, k:k + 1]
            xn = s["xn"][it]
            nc.vector.tensor_scalar_mul(out=xn, in0=xn, scalar1=rs)
            for eg in range(2):
                tp = ps_small.tile([P, 4, P], F32, tag="smallps")
                for kk in range(4):
                    et = 4 * eg + kk
                    nc.tensor.matmul(
                        tp[:, kk, :], xn[:, et * P:(et + 1) * P], id_sb,
                        start=True, stop=True)
                nc.vector.tensor_copy(
                    out=xT[:, 4 * eg:4 * eg + 4, it * P:(it + 1) * P],
                    in_=tp)

        def emit_qk_chunk(b, t, nt):
            # compact q^T/k^T halves; on the last nt of each t, relocate
            # head rows into the 32-aligned region layout.
            s = st8[b]
            if s.get("qkc") is None:
                s["qkc"] = qk_pool.tile([P, 2, N], BF16, tag="qkc", name="qkc")
                s["qT"] = qk_pool.tile([P, 2, N], BF16, tag="qT", name="qT")
                s["kT"] = qk_pool.tile([P, 2, N], BF16, tag="kT", name="kT")
            xT = s["xT"]
            ps = ps_small.tile([P, 512], F32, tag="smallps")
            for et in range(NT):
                nc.tensor.matmul(
                    ps, wqk_sb[:, et, t, :],
                    xT[:, et, nt * 512:(nt + 1) * 512],
                    start=(et == 0), stop=(et == NT - 1))
            nc.vector.tensor_scalar(
                out=s["qkc"][:, t, nt * 512:(nt + 1) * 512], in0=ps,
                scalar1=bqk_sb[:, t:t + 1], scalar2=None, op0=Add)
            if nt == 1:
                dst = s["qT"] if t == 0 else s["kT"]
                eng = nc.scalar if b == 0 else nc.sync
                for h in range(HEADS):
                    r, c = h // 4, h % 4
                    eng.dma_start(
                        out=dst[32 * c:32 * c + HD, r, :],
                        in_=s["qkc"][HD * h:HD * (h + 1), t, :])

        def emit_v_chunk(b, jt):
            s = st8[b]
            if s.get("v") is None:
                s["v"] = v_pool.tile([P, NT, HEADS, 32], BF16, tag="vt", name="vt")
                nc.gpsimd.memset(s["v"], 0.0)
                nc.gpsimd.memset(s["v"][:, :, :, 0:1], 1.0)
            xT = s["xT"]
            ps = ps_small.tile([P, P], F32, tag="smallps")
            for et in range(NT):
                nc.tensor.matmul(
                    ps, xT[:, et, jt * P:(jt + 1) * P], wv_sb[:, et, :],
                    start=(et == 0), stop=False)
            nc.tensor.matmul(ps, ones1_sb, bv_sb, start=False, stop=True)
            nc.vector.tensor_copy(
                out=s["v"][:, jt, :, 1:17],
                in_=ps[:].rearrange("p (h d) -> p h d", d=16))

        def emit_proj_chunk(b, it, nt):
            s = st8[b]
            ps = ps_small.tile([P, 512], F32, tag="smallps")
            for r in range(2):
                nc.tensor.matmul(
                    ps, s["o"][r][:, it * P:(it + 1) * P],
                    wpj_sb[:, r, nt * 512:(nt + 1) * 512],
                    start=(r == 0), stop=(r == 1))
            fin = fin_pool.tile([P, 512], F32, tag="fin")
            nc.vector.tensor_copy(out=fin, in_=ps)
            nc.sync.dma_start(
                out=out_h[b, it * P:(it + 1) * P, nt * 512:(nt + 1) * 512],
                in_=fin)

        def emit_proj1(b, it, nt):
            # region-0 half of the projection, stashed in SBUF f32
            s = st8[b]
            if s.get("fin1") is None:
                s["fin1"] = fin_pool.tile([P, NT, 2, 512], F32,
                                          tag="fin1", name="fin1", bufs=1)
            ps = ps_small.tile([P, 512], F32, tag="smallps")
            nc.tensor.matmul(
                ps, s["o"][0][:, it * P:(it + 1) * P],
                wpj_sb[:, 0, nt * 512:(nt + 1) * 512],
                start=True, stop=True)
            nc.vector.tensor_copy(out=s["fin1"][:, it, nt, :], in_=ps)

        def emit_proj2(b, it, nt):
            s = st8[b]
            ps = ps_small.tile([P, 512], F32, tag="smallps")
            nc.tensor.matmul(
                ps, s["o"][1][:, it * P:(it + 1) * P],
                wpj_sb[:, 1, nt * 512:(nt + 1) * 512],
                start=True, stop=True)
            fin = fin_pool.tile([P, 512], F32, tag="fin")
            nc.vector.tensor_add(fin, s["fin1"][:, it, nt, :], ps)
            nc.sync.dma_start(
                out=out_h[b, it * P:(it + 1) * P, nt * 512:(nt + 1) * 512],
                in_=fin)

        def emit_normalize(b, r, ih, oT_ps):
            s = st8[b]
            if s["o"][r] is None:
                s["o"][r] = o_pool.tile([P, N], BF16, tag="oT", name="oT")
            i0 = ih * 512
            srow = nrm_pool.tile([P, 512], F32, tag="srow")
            for c in range(4):
                nc.vector.tensor_copy(
                    out=srow[32 * c:32 * c + 1, :],
                    in_=oT_ps[32 * c:32 * c + 1, :])
            scr1 = dram_pool.tile([4, 512], F32, tag="scr1")
            nc.sync.dma_start(out=scr1, in_=srow[0::32, :])
            cmp = nrm_pool.tile([P, 16], F32, tag="cmp")
            flat = scr1[:].rearrange("a (pp cc) -> (a pp) cc", cc=16)
            nc.sync.dma_start(out=cmp, in_=flat)
            rec = nrm_pool.tile([P, 16], F32, tag="rec")
            nc.vector.reciprocal(out=rec, in_=cmp)
            scr2 = dram_pool.tile([4, 512], F32, tag="scr2")
            nc.sync.dma_start(
                out=scr2[:].rearrange("a (pp cc) -> (a pp) cc", cc=16),
                in_=rec)
            rep = nrm_pool.tile([P, 512], F32, tag="rep")
            for c in range(4):
                src = scr2[c:c + 1, :]
                bcast = bass.AP(
                    tensor=src.tensor, offset=src.offset,
                    ap=[[0, 32]] + list(src.ap[1:]))
                nc.sync.dma_start(
                    out=rep[32 * c:32 * c + 32, :], in_=bcast)
            nc.vector.tensor_mul(s["o"][r][:, i0:i0 + 512], oT_ps, rep)

        def emit_attention(b, fillers, rate=2):
            s = st8[b]
            s["o"] = [None, None]
            slot = [0]

            def maybe_fill():
                slot[0] += 1
                if fillers and (rate > 0 and slot[0] % rate == 0 or
                                (rate == 0 and (slot[0] % 2 == 0 or
                                                slot[0] > 32))):
                    f = fillers.pop(0)
                    if f is not None:
                        f()

            for r in range(2):
                for ih in range(2):
                    oT_ps = ps_out.tile([P, 512], F32, tag="oTps")
                    i0 = ih * 512
                    for cp in range(2):
                        c0 = 2 * cp
                        for jt in range(NT):
                            E = e_pool.tile([P, 2, 512], BF16, tag="E")
                            sc = ps_scores.tile([P, 2, 512], F32, tag="sc")
                            for ci in range(2):
                                c = c0 + ci
                                nc.tensor.matmul(
                                    sc[:, ci, :],
                                    s["kT"][32 * c:32 * c + 16, r,
                                            jt * P:(jt + 1) * P],
                                    s["qT"][32 * c:32 * c + 16, r,
                                            i0:i0 + 512],
                                    start=True, stop=True,
                                    tile_position=(32 * c, 0))
                            nc.scalar.activation(out=E, in_=sc, func=AF.Exp)
                            for ci in range(2):
                                c = c0 + ci
                                h = 4 * r + c
                                nc.tensor.matmul(
                                    oT_ps[32 * c:32 * c + 32, :],
                                    s["v"][:, jt, h, :], E[:, ci, :],
                                    start=(jt == 0), stop=(jt == NT - 1),
                                    tile_position=(0, 32 * c))
                            maybe_fill()
                    emit_normalize(b, r, ih, oT_ps)

        # ---------- schedule ----------
        st8[0] = {}
        st8[1] = {}
        # preload the exp table while the ramp runs
        dummy = stat.tile([P, 1], F32, tag="dummy")
        nc.scalar.activation(out=dummy, in_=eps_sb, func=AF.Exp)

        def ab_order(b):
            out = []
            for it in range(4):
                out.append(lambda it=it: emit_ln_pre(b, it))
            out.append(lambda: emit_newton(b, 0))
            for it in range(4):
                out.append(lambda it=it: emit_ln_post(b, it))
                out.append(lambda it=it: emit_v_chunk(b, it))
            out.append(lambda: emit_qk_chunk(b, 0, 0))
            out.append(lambda: emit_qk_chunk(b, 1, 0))
            for it in range(4, NT):
                out.append(lambda it=it: emit_ln_pre(b, it))
            out.append(lambda: emit_newton(b, 1))
            for it in range(4, NT):
                out.append(lambda it=it: emit_ln_post(b, it))
                out.append(lambda it=it: emit_v_chunk(b, it))
            out.append(lambda: emit_qk_chunk(b, 0, 1))
            out.append(lambda: emit_qk_chunk(b, 1, 1))
            return out

        for f in ab_order(0):
            f()

        fill_b1 = ab_order(1)
        emit_attention(0, fill_b1)
        for f in fill_b1:
            f()

        fill_a1 = (
            [lambda it=it, nt=nt: emit_proj_chunk(0, it, nt)
             for it in range(NT) for nt in range(2)]          # pops 2..32
            + [lambda it=it, nt=nt: emit_proj1(1, it, nt)
               for it in range(4) for nt in range(2)]         # 33-40
            + [lambda it=it, nt=nt: emit_proj1(1, it, nt)
               for it in range(4, NT) for nt in range(2)]     # 41-48
            + [lambda it=it, nt=nt: emit_proj2(1, it, nt)
               for it in range(4) for nt in range(2)]         # 49-56
        )
        emit_attention(1, fill_a1, rate=0)
        for f in fill_a1:
            if f is not None:
                f()
        for it in range(4, NT):
            for nt in range(2):
                emit_proj2(1, it, nt)

    nc.finalize()
    return nc
def _prep_weights(gamma, beta, w_qkv, w_proj, b_proj):
    gamma = gamma.astype(np.float64)
    beta = beta.astype(np.float64)
    w_qkv = w_qkv.astype(np.float64)
    w_proj = w_proj.astype(np.float64)
    b_proj = b_proj.astype(np.float64)

    wg = w_qkv * gamma[:, None]
    bias = beta @ w_qkv                   # [384]

    # compact q/k: tile t=0 -> q (SCALE folded), t=1 -> k
    wqk = np.zeros((EMB, 2, P), dtype=np.float64)
    wqk[:, 0, :] = wg[:, :INNER] * SCALE
    wqk[:, 1, :] = wg[:, INNER:2 * INNER]
    bqk = np.zeros((P, 2), dtype=np.float64)
    bqk[:, 0] = bias[:INNER] * SCALE
    bqk[:, 1] = bias[INNER:2 * INNER]
    wqk_t = wqk.reshape(NT, P, 2, P).transpose(1, 0, 2, 3)  # [P, NT, 2, P]

    wv = wg[:, 2 * INNER:3 * INNER].reshape(NT, P, P).transpose(1, 0, 2)
    bv = bias[2 * INNER:3 * INNER].reshape(1, P)

    # o^T row mapping: 32c = ones/rowsum row, 32c+1+d = head (4r+c) dim d
    wpj = np.zeros((P, 2, EMB), dtype=np.float64)
    for r in range(2):
        for c in range(4):
            h = 4 * r + c
            wpj[32 * c + 1:32 * c + 1 + HD, r, :] = \
                w_proj[h * HD:(h + 1) * HD, :]
    wpj[0, 0, :] = b_proj

    bf = ml_dtypes.bfloat16
    return {
        "wqk": np.ascontiguousarray(wqk_t).astype(bf),
        "bqk": np.ascontiguousarray(bqk).astype(np.float32),
        "wv": np.ascontiguousarray(wv).astype(bf),
        "bv": np.ascontiguousarray(bv).astype(bf),
        "wproj": np.ascontiguousarray(wpj).astype(bf),
        "ident": np.eye(P, dtype=np.float32).astype(bf),
    }


def kernel(x, gamma, beta, w_qkv, w_proj, b_proj):
    if "nc" not in _CACHE:
        _CACHE["nc"] = _build()
    nc = _CACHE["nc"]

    w = _prep_weights(gamma, beta, w_qkv, w_proj, b_proj)
    x = np.asarray(x, dtype=np.float32)
    in_maps = []
    for i in range(NCORES):
        m = {"xs": np.ascontiguousarray(x[i * NB:(i + 1) * NB])}
        m.update(w)
        in_maps.append(m)

    res = run_bass_kernel_spmd(nc, in_maps, core_ids=list(range(NCORES)))
    out = np.concatenate([res.results[i]["out"] for i in range(NCORES)], axis=0)
    return out.astype(np.float32)
